# revision 1
# baseline (speedup 1.0000x reference)
"""Trainium2 Bass kernel for nn_CustomAttn: qkv proj + flat-axis qk-RMSnorm +
RoPE + causal attention + out proj, tensor-parallel over heads (Megatron-style)
with data-parallel batch, on 8 NeuronCores.

Mesh: core c -> batch b = c // 4, head-group hg = c % 4 (heads hg*4 .. hg*4+3).
Groups [[0,1,2,3],[4,5,6,7]]: AllReduce for the qk-norm sum-of-squares (the
norm is over the flat 2048-dim axis, i.e. across all 16 heads), and per
512-token-tile ReduceScatter for the output-projection partial sums (fires as
soon as that tile's attention + projection are done, overlapping comm with the
next tile's compute).  Core c holds rows j*128..(j+1)*128 of out-tile j, which
the host maps to tokens j*512 + (c%4)*128.
"""

import sys

for p in ("/opt/trn_rl_repo",):
    if p not in sys.path:
        sys.path.insert(0, p)

import numpy as np
import ml_dtypes
from contextlib import ExitStack

import concourse.bass as bass
import concourse.bacc as bacc
from concourse.tile import TileContext
from concourse import mybir
from concourse.bass_utils import run_bass_kernel_spmd

BF16 = mybir.dt.bfloat16
F32 = mybir.dt.float32
NPBF16 = ml_dtypes.bfloat16

B, S, HID = 2, 2048, 2048
NH, HD = 16, 128
EPS = 1e-5
ROPE_BASE = 10000.0

NCORES = 8
TPG = 4                    # tensor-parallel group size
NHL = NH // TPG            # 4 local heads
DL = NHL * HD              # 512 local q/k/v dims
KT = HID // 128            # 16 contraction chunks
NT = S // 512              # 4 token tiles of 512
TB = S // 128              # 16 token blocks of 128
GROUPS = [[0, 1, 2, 3], [4, 5, 6, 7]]
SCALE = 1.0 / float(np.sqrt(HD))

LAST_EXEC_NS = None
_CACHED_NC = None


def build_nc():
    nc = bacc.Bacc(num_devices=NCORES)

    xT = nc.declare_dram_parameter("xT", [HID, S], BF16, isOutput=False)
    w_inT = nc.declare_dram_parameter("w_inT", [HID, 3 * DL], BF16, isOutput=False)
    w_outT = nc.declare_dram_parameter("w_outT", [DL, HID], BF16, isOutput=False)
    qn = nc.declare_dram_parameter("qn", [128, 4], F32, isOutput=False)
    kn = nc.declare_dram_parameter("kn", [128, 4], F32, isOutput=False)
    c128 = nc.declare_dram_parameter("c128", [128, S], BF16, isOutput=False)
    s128 = nc.declare_dram_parameter("s128", [128, S], BF16, isOutput=False)
    maskT = nc.declare_dram_parameter("maskT", [128, 128], BF16, isOutput=False)
    out = nc.declare_dram_parameter("out", [512, HID], F32, isOutput=True)

    cc_in = nc.dram_tensor("cc_in", [2, S], F32)
    cc_out = nc.dram_tensor("cc_out", [2, S], F32)
    op_buf = nc.dram_tensor("op_buf", [NT, 512, HID], BF16)
    rs_out = nc.dram_tensor("rs_out", [NT, 128, HID], BF16)

    with TileContext(nc) as tc, ExitStack() as ctx:
        consts = ctx.enter_context(tc.tile_pool(name="consts", bufs=1))
        weights = ctx.enter_context(tc.tile_pool(name="weights", bufs=1))
        persist = ctx.enter_context(tc.tile_pool(name="persist", bufs=1))
        xpool = ctx.enter_context(tc.tile_pool(name="xpool", bufs=2))
        sqp = ctx.enter_context(tc.tile_pool(name="sqp", bufs=2))
        mmp = ctx.enter_context(tc.tile_pool(name="mmp", bufs=3, space="PSUM"))
        accp = ctx.enter_context(tc.tile_pool(name="accp", bufs=2, space="PSUM"))
        smallp = ctx.enter_context(tc.tile_pool(name="smallp", bufs=2, space="PSUM"))
        ropet = ctx.enter_context(tc.tile_pool(name="ropet", bufs=1))
        rqp = ctx.enter_context(tc.tile_pool(name="rqp", bufs=2))
        attnp = ctx.enter_context(tc.tile_pool(name="attnp", bufs=2))
        expp = ctx.enter_context(tc.tile_pool(name="expp", bufs=4))
        wop = ctx.enter_context(tc.tile_pool(name="wop", bufs=2))
        outp = ctx.enter_context(tc.tile_pool(name="outp", bufs=1))

        # --- constants ---
        ones_col = consts.tile([128, 1], F32)          # lhsT for partition-sum (f32)
        nc.vector.memset(ones_col, 1.0)
        ones_col_b = consts.tile([128, 1], BF16)       # bf16 ones for denom/ssq
        nc.vector.memset(ones_col_b, 1.0)
        ones_row = consts.tile([1, 128], F32)          # lhsT for partition bcast
        nc.vector.memset(ones_row, 1.0)
        ones_row_b = consts.tile([1, 128], BF16)
        nc.vector.memset(ones_row_b, 1.0)
        qn_t = consts.tile([128, 4], F32)
        nc.sync.dma_start(out=qn_t, in_=qn[:, :])
        kn_t = consts.tile([128, 4], F32)
        nc.sync.dma_start(out=kn_t, in_=kn[:, :])
        mask_t = consts.tile([128, 128], BF16)
        nc.sync.dma_start(out=mask_t, in_=maskT[:, :])
        zeros_b = consts.tile([128, 1], F32)           # explicit bias for Exp
        nc.vector.memset(zeros_b, 0.0)
        eps_b = consts.tile([1, 1], F32)               # explicit bias for Sqrt
        nc.vector.memset(eps_b, EPS)

        # --- resident weights ---
        w_tiles = []
        for k in range(KT):
            wt = weights.tile([128, 3 * DL], BF16, tag=f"w{k}", name=f"w{k}")
            nc.sync.dma_start(out=wt, in_=w_inT[k * 128:(k + 1) * 128, :])
            w_tiles.append(wt)

        # q/k staging (rope applied in place later): m 0..3 = q chunks,
        # m 4..7 = k chunks, each [128 dims, S tokens] bf16
        qk_tiles = [persist.tile([128, S], BF16, tag=f"qk{m}", name=f"qk{m}")
                    for m in range(8)]
        # v in token-major layout: [128 tokens, 512 vdims] per token block
        v_tiles = [persist.tile([128, DL], BF16, tag=f"v{tb}", name=f"v{tb}")
                   for tb in range(TB)]

        # ---------- phase 1: q/k projection + sum-of-squares ----------
        saved_xt = {}
        for n in range(NT):
            xt = []
            for k in range(KT):
                t = xpool.tile([128, 512], BF16, tag=f"x{k}", name=f"x{k}")
                nc.sync.dma_start(
                    out=t, in_=xT[k * 128:(k + 1) * 128, n * 512:(n + 1) * 512])
                xt.append(t)

            for ti, base in ((0, 0), (1, 4)):       # q then k chunks
                ssq_ps = smallp.tile([1, 512], F32, tag="ssq")
                for mi in range(4):
                    m = base + mi
                    pq = mmp.tile([128, 512], F32, tag="mm")
                    for k in range(KT):
                        nc.tensor.matmul(
                            pq, w_tiles[k][:, m * 128:(m + 1) * 128], xt[k],
                            start=(k == 0), stop=(k == KT - 1))
                    sq = sqp.tile([128, 512], BF16, tag="sq")
                    nc.scalar.square(sq, pq)
                    nc.tensor.matmul(ssq_ps, ones_col_b, sq,
                                     start=(mi == 0), stop=(mi == 3))
                    ncol = (qn_t if ti == 0 else kn_t)[:, mi:mi + 1]
                    nc.scalar.mul(qk_tiles[m][:, n * 512:(n + 1) * 512], pq, ncol)
                ssq_s = sqp.tile([1, 512], F32, tag="invd", name="ssq_s", bufs=2)
                nc.scalar.copy(ssq_s, ssq_ps)
                nc.sync.dma_start(
                    out=cc_in[ti:ti + 1, n * 512:(n + 1) * 512], in_=ssq_s)

            if n < 2:                               # v-proj inline (x resident)
                for tbl in range(4):
                    tb = n * 4 + tbl
                    pv = mmp.tile([128, 512], F32, tag="mm")
                    for k in range(KT):
                        nc.tensor.matmul(
                            pv, xt[k][:, tbl * 128:(tbl + 1) * 128],
                            w_tiles[k][:, 2 * DL:3 * DL],
                            start=(k == 0), stop=(k == KT - 1))
                    nc.vector.tensor_copy(v_tiles[tb], pv)
            else:                                   # keep x for post-AR v-proj
                saved_xt[n] = xt

        # ---------- phase 2: allreduce sumsq; v-proj of tiles 2,3 runs under
        # it using the still-resident x tiles (xpool bufs=2 holds both) ------
        nc.gpsimd.collective_compute(
            "AllReduce", mybir.AluOpType.add, replica_groups=GROUPS,
            ins=[cc_in[:, :]], outs=[cc_out[:, :]])

        for n in (2, 3):
            xt = saved_xt[n]
            for tbl in range(4):
                tb = n * 4 + tbl
                pv = mmp.tile([128, 512], F32, tag="mm")
                for k in range(KT):
                    nc.tensor.matmul(
                        pv, xt[k][:, tbl * 128:(tbl + 1) * 128],
                        w_tiles[k][:, 2 * DL:3 * DL],
                        start=(k == 0), stop=(k == KT - 1))
                nc.vector.tensor_copy(v_tiles[tb], pv)

        # ---------- phase 3: inv_rms + rope tables + rope ----------
        inv_t = []
        for t in range(2):
            tot = persist.tile([1, S], F32, tag=f"tot{t}", name=f"tot{t}")
            nc.sync.dma_start(out=tot, in_=cc_out[t:t + 1, :])
            nc.scalar.activation(tot, tot, mybir.ActivationFunctionType.Sqrt,
                                 bias=eps_b, scale=1.0 / (NH * HD))
            nc.vector.reciprocal(tot, tot)
            inv_t.append(tot)

        ci = [persist.tile([128, S], BF16, tag=f"ci{t}", name=f"ci{t}")
              for t in range(2)]
        si = [persist.tile([128, S], BF16, tag=f"si{t}", name=f"si{t}")
              for t in range(2)]
        for t in range(2):
            for j in range(NT):
                sl = slice(j * 512, (j + 1) * 512)
                bc = mmp.tile([128, 512], F32, tag="mm")
                nc.tensor.matmul(bc, ones_row, inv_t[t][:, sl],
                                 start=True, stop=True)
                cs = ropet.tile([128, 512], BF16, tag="rt", name="cs")
                nc.sync.dma_start(out=cs, in_=c128[:, sl])
                nc.vector.tensor_mul(ci[t][:, sl], cs, bc)
                ss = ropet.tile([128, 512], BF16, tag="rt2", name="ss")
                nc.sync.dma_start(out=ss, in_=s128[:, sl])
                nc.vector.tensor_mul(si[t][:, sl], ss, bc)

        # rope in place: q' = q*ci + rot(q)*si  (rot = half-swap via DMA,
        # si carries the [-sin; sin] signs)
        for j in range(NT):
            for m in range(8):
                t = 0 if m < 4 else 1
                qk = qk_tiles[m]
                sl = slice(j * 512, (j + 1) * 512)
                rq = rqp.tile([128, 512], BF16, tag="rq", name="rq")
                nc.sync.dma_start(out=rq[0:64, :], in_=qk[64:128, sl])
                nc.sync.dma_start(out=rq[64:128, :], in_=qk[0:64, sl])
                nc.vector.tensor_mul(qk[:, sl], qk[:, sl], ci[t][:, sl])
                nc.vector.tensor_mul(rq, rq, si[t][:, sl])
                nc.vector.tensor_add(qk[:, sl], qk[:, sl], rq)

        # ---------- phase 4+5: attention (j outer) + out proj + RS per tile --
        w_out_tiles = []
        for h in range(NHL):
            wt = persist.tile([128, HID], BF16, tag=f"wo{h}", name=f"wot{h}")
            nc.sync.dma_start(out=wt, in_=w_outT[h * 128:(h + 1) * 128, :])
            w_out_tiles.append(wt)

        for j in range(NT):
            attn_j = []
            for h in range(NHL):
                kt_h, qt_h = qk_tiles[4 + h], qk_tiles[h]
                pv_ps = accp.tile([128, 512], F32, tag="pv")
                den_ps = smallp.tile([1, 512], F32, tag="ssq")
                nb = 4 * j + 4
                for b in range(nb):
                    r = b - 4 * j
                    q_off = max(r, 0) * 128
                    w = 512 - q_off
                    s_ps = mmp.tile([128, 512], F32, tag="mm")
                    nc.tensor.matmul(
                        s_ps[:, :w], kt_h[:, b * 128:(b + 1) * 128],
                        qt_h[:, j * 512 + q_off:(j + 1) * 512],
                        start=True, stop=True)
                    ex = expp.tile([128, 512], BF16, tag="exp")
                    nc.scalar.activation(ex[:, :w], s_ps[:, :w],
                                         mybir.ActivationFunctionType.Exp,
                                         bias=zeros_b, scale=SCALE)
                    if r >= 0:
                        nc.vector.tensor_mul(ex[:, 0:128], ex[:, 0:128], mask_t)
                    nc.tensor.matmul(
                        pv_ps[:, q_off:512],
                        v_tiles[b][:, h * 128:(h + 1) * 128], ex[:, :w],
                        start=(b == 0), stop=(b == nb - 1))
                    nc.tensor.matmul(
                        den_ps[0:1, q_off:512], ones_col_b, ex[:, :w],
                        start=(b == 0), stop=(b == nb - 1))
                inv_d = sqp.tile([1, 512], BF16, tag="invd", bufs=2)
                with nc.allow_low_precision(reason="softmax denom bcast in bf16"):
                    nc.vector.reciprocal(inv_d, den_ps)
                bc = mmp.tile([128, 512], F32, tag="mm")
                nc.tensor.matmul(bc, ones_row_b, inv_d, start=True, stop=True)
                bc_sb = sqp.tile([128, 512], F32, tag="bcsb", name="bc_sb",
                                 bufs=1)
                nc.scalar.copy(bc_sb, bc)
                at = attnp.tile([128, 512], BF16, tag=f"at{h}", name=f"at{h}")
                nc.vector.tensor_mul(at, pv_ps, bc_sb)
                attn_j.append(at)

            # out projection for this token tile, then reduce-scatter it
            for tbl in range(4):
                for c in range(4):
                    po = mmp.tile([128, 512], F32, tag="mm")
                    for h in range(NHL):
                        nc.tensor.matmul(
                            po, attn_j[h][:, tbl * 128:(tbl + 1) * 128],
                            w_out_tiles[h][:, c * 512:(c + 1) * 512],
                            start=(h == 0), stop=(h == NHL - 1))
                    ws = wop.tile([128, 512], BF16, tag="wo")
                    nc.vector.tensor_copy(ws, po)
                    nc.sync.dma_start(
                        out=op_buf[j, tbl * 128:(tbl + 1) * 128,
                                   c * 512:(c + 1) * 512], in_=ws)
            nc.gpsimd.collective_compute(
                "ReduceScatter", mybir.AluOpType.add, replica_groups=GROUPS,
                ins=[op_buf[j]], outs=[rs_out[j]])

        # ---------- phase 6: gather shard, convert to f32, store ----------
        # core's rows j*128..(j+1)*128 of `out` = its quarter of out-tile j
        for j in range(NT):
            for c in range(4):
                t = outp.tile([128, 512], BF16, tag="fin_b")
                nc.sync.dma_start(
                    out=t, in_=rs_out[j, :, c * 512:(c + 1) * 512])
                t32 = outp.tile([128, 512], F32, tag="fin_f")
                nc.vector.tensor_copy(t32, t)
                nc.sync.dma_start(
                    out=out[j * 128:(j + 1) * 128, c * 512:(c + 1) * 512],
                    in_=t32)

    nc.finalize()
    return nc


def make_in_maps(x, w_in, w_out, q_norm_w, k_norm_w):
    x = np.asarray(x, np.float32)
    w_in = np.asarray(w_in, np.float32)
    w_out = np.asarray(w_out, np.float32)
    q_norm_w = np.asarray(q_norm_w, np.float32)
    k_norm_w = np.asarray(k_norm_w, np.float32)

    half = HD // 2
    inv_freq = 1.0 / (ROPE_BASE ** (np.arange(half, dtype=np.float32) / half))
    pos = np.arange(S, dtype=np.float32)
    ang = pos[:, None] * inv_freq[None, :]              # [S, 64]
    cos = np.cos(ang).T                                 # [64, S]
    sin = np.sin(ang).T
    c128 = np.concatenate([cos, cos], axis=0).astype(NPBF16)   # [128, S]
    s128 = np.concatenate([-sin, sin], axis=0).astype(NPBF16)  # rotate-half signs
    maskT = (np.arange(128)[:, None] <= np.arange(128)[None, :]).astype(NPBF16)

    in_maps = []
    for c in range(NCORES):
        b, hg = c // TPG, c % TPG
        rows = np.concatenate([
            w_in[hg * DL:(hg + 1) * DL],
            w_in[NH * HD + hg * DL:NH * HD + (hg + 1) * DL],
            w_in[2 * NH * HD + hg * DL:2 * NH * HD + (hg + 1) * DL],
        ], axis=0)                                      # [1536, HID]
        in_maps.append({
            "xT": np.ascontiguousarray(x[b].T).astype(NPBF16),
            "w_inT": np.ascontiguousarray(rows.T).astype(NPBF16),
            "w_outT": np.ascontiguousarray(
                w_out[:, hg * DL:(hg + 1) * DL].T).astype(NPBF16),
            "qn": np.ascontiguousarray(
                q_norm_w[hg * DL:(hg + 1) * DL].reshape(4, 128).T),
            "kn": np.ascontiguousarray(
                k_norm_w[hg * DL:(hg + 1) * DL].reshape(4, 128).T),
            "c128": c128, "s128": s128, "maskT": maskT,
        })
    return in_maps


def assemble(results):
    """results[c] is [512, HID]: rows j*128..(j+1)*128 are this core's rank
    slice of token tile j."""
    outp = np.empty((B, S, HID), np.float32)
    for c in range(NCORES):
        b, t = c // TPG, c % TPG
        r = np.asarray(results[c], np.float32)
        for j in range(NT):
            outp[b, j * 512 + t * 128:j * 512 + (t + 1) * 128, :] = \
                r[j * 128:(j + 1) * 128, :]
    return outp


def kernel(x, w_in, w_out, q_norm_w, k_norm_w, trace=False):
    global LAST_EXEC_NS, _CACHED_NC
    if _CACHED_NC is None:
        _CACHED_NC = build_nc()
    nc = _CACHED_NC
    in_maps = make_in_maps(x, w_in, w_out, q_norm_w, k_norm_w)
    res = run_bass_kernel_spmd(nc, in_maps, list(range(NCORES)), trace=trace)
    LAST_EXEC_NS = res.exec_time_ns
    return assemble([res.results[c]["out"] for c in range(NCORES)])



# revision 19
# speedup vs baseline: 1.1877x; 1.1877x over previous
"""Trainium2 Bass kernel for nn_CustomAttn: qkv proj + flat-axis qk-RMSnorm +
RoPE + causal attention + out proj, tensor-parallel over heads (Megatron-style)
with data-parallel batch, on 8 NeuronCores.

Mesh: core c -> batch b = c // 4, head-group hg = c % 4 (heads hg*4 .. hg*4+3).
Groups [[0,1,2,3],[4,5,6,7]]: AllReduce for the qk-norm sum-of-squares (the
norm is over the flat 2048-dim axis, i.e. across all 16 heads), and per
512-token-tile AllGather of the attention outputs (each rank contributes its
512 head-dims; 4x less effective wire time than ReduceScatter of 2 MB
partials).  After the gather every core holds the full [2048 dims, 512 tok]
attention block and runs the output projection over the full contraction for
ITS 512 output-feature columns only (w_out column-sliced by the host, so the
NEFF stays rank-symmetric), accumulating in f32.  Core c's `out` is
[S, 512] f32 = output features hg*512..(hg+1)*512 for all tokens of batch b.
"""

import sys

for p in ("/opt/trn_rl_repo",):
    if p not in sys.path:
        sys.path.insert(0, p)

import numpy as np
import ml_dtypes
from contextlib import ExitStack

import concourse.bass as bass
import concourse.bacc as bacc
from concourse.tile import TileContext
from concourse import mybir
from concourse.bass_utils import run_bass_kernel_spmd

BF16 = mybir.dt.bfloat16
F32 = mybir.dt.float32
NPBF16 = ml_dtypes.bfloat16

B, S, HID = 2, 2048, 2048
NH, HD = 16, 128
EPS = 1e-5
ROPE_BASE = 10000.0

NCORES = 8
TPG = 4                    # tensor-parallel group size
NHL = NH // TPG            # 4 local heads
DL = NHL * HD              # 512 local q/k/v dims
KT = HID // 128            # 16 contraction chunks
NT = S // 512              # 4 token tiles of 512
TB = S // 128              # 16 token blocks of 128
GROUPS = [[0, 1, 2, 3], [4, 5, 6, 7]]
SCALE = 1.0 / float(np.sqrt(HD))

LAST_EXEC_NS = None
_CACHED_NC = None


def build_nc():
    nc = bacc.Bacc(num_devices=NCORES)

    xT = nc.declare_dram_parameter("xT", [HID, S], BF16, isOutput=False)
    w_inT = nc.declare_dram_parameter("w_inT", [HID, 3 * DL], BF16, isOutput=False)
    w_outT = nc.declare_dram_parameter("w_outT", [NH * HD, DL], BF16, isOutput=False)
    qn = nc.declare_dram_parameter("qn", [128, 4], F32, isOutput=False)
    kn = nc.declare_dram_parameter("kn", [128, 4], F32, isOutput=False)
    c128 = nc.declare_dram_parameter("c128", [128, S], BF16, isOutput=False)
    s128 = nc.declare_dram_parameter("s128", [128, S], BF16, isOutput=False)
    maskT = nc.declare_dram_parameter("maskT", [128, 128], BF16, isOutput=False)
    out = nc.declare_dram_parameter("out", [S, DL], F32, isOutput=True)

    cc_in = nc.dram_tensor("cc_in", [2, S], F32)
    cc_out = nc.dram_tensor("cc_out", [2, S], F32)
    ag_in = nc.dram_tensor("ag_in", [NT, DL, 512], BF16)
    ag_out = nc.dram_tensor("ag_out", [NT, NH * HD, 512], BF16)

    with TileContext(nc) as tc, ExitStack() as ctx:
        consts = ctx.enter_context(tc.tile_pool(name="consts", bufs=1))
        weights = ctx.enter_context(tc.tile_pool(name="weights", bufs=1))
        persist = ctx.enter_context(tc.tile_pool(name="persist", bufs=1))
        xpool = ctx.enter_context(tc.tile_pool(name="xpool", bufs=2))
        sqp = ctx.enter_context(tc.tile_pool(name="sqp", bufs=2))
        mmp = ctx.enter_context(tc.tile_pool(name="mmp", bufs=3, space="PSUM"))
        accp = ctx.enter_context(tc.tile_pool(name="accp", bufs=2, space="PSUM"))
        smallp = ctx.enter_context(tc.tile_pool(name="smallp", bufs=2, space="PSUM"))
        ropet = ctx.enter_context(tc.tile_pool(name="ropet", bufs=1))
        rqp = ctx.enter_context(tc.tile_pool(name="rqp", bufs=2))
        attnp = ctx.enter_context(tc.tile_pool(name="attnp", bufs=2))
        expp = ctx.enter_context(tc.tile_pool(name="expp", bufs=4))
        agp = ctx.enter_context(tc.tile_pool(name="agp", bufs=1))
        outp = ctx.enter_context(tc.tile_pool(name="outp", bufs=2))

        # --- constants ---
        ones_col = consts.tile([128, 1], F32)          # lhsT for partition-sum (f32)
        nc.vector.memset(ones_col, 1.0)
        ones_col_b = consts.tile([128, 1], BF16)       # bf16 ones for denom/ssq
        nc.vector.memset(ones_col_b, 1.0)
        ones_row = consts.tile([1, 128], F32)          # lhsT for partition bcast
        nc.vector.memset(ones_row, 1.0)
        ones_row_b = consts.tile([1, 128], BF16)
        nc.vector.memset(ones_row_b, 1.0)
        qn_t = consts.tile([128, 4], F32)
        nc.sync.dma_start(out=qn_t, in_=qn[:, :])
        kn_t = consts.tile([128, 4], F32)
        nc.sync.dma_start(out=kn_t, in_=kn[:, :])
        mask_t = consts.tile([128, 128], BF16)
        nc.sync.dma_start(out=mask_t, in_=maskT[:, :])
        zeros_b = consts.tile([128, 1], F32)           # explicit bias for Exp
        nc.vector.memset(zeros_b, 0.0)
        eps_b = consts.tile([1, 1], F32)               # explicit bias for Sqrt
        nc.vector.memset(eps_b, EPS)

        # --- resident weights ---
        w_tiles = []
        for k in range(KT):
            wt = weights.tile([128, 3 * DL], BF16, tag=f"w{k}", name=f"w{k}")
            nc.sync.dma_start(out=wt, in_=w_inT[k * 128:(k + 1) * 128, :])
            w_tiles.append(wt)

        # q/k staging (rope applied in place later): m 0..3 = q chunks,
        # m 4..7 = k chunks, each [128 dims, S tokens] bf16
        qk_tiles = [persist.tile([128, S], BF16, tag=f"qk{m}", name=f"qk{m}")
                    for m in range(8)]
        # v in token-major layout: [128 tokens, 512 vdims] per token block
        v_tiles = [persist.tile([128, DL], BF16, tag=f"v{tb}", name=f"v{tb}")
                   for tb in range(TB)]

        # ---------- phase 1: q/k projection + sum-of-squares ----------
        saved_xt = {}
        for n in range(NT):
            xt = []
            for k in range(KT):
                t = xpool.tile([128, 512], BF16, tag=f"x{k}", name=f"x{k}")
                nc.sync.dma_start(
                    out=t, in_=xT[k * 128:(k + 1) * 128, n * 512:(n + 1) * 512])
                xt.append(t)

            for ti, base in ((0, 0), (1, 4)):       # q then k chunks
                ssq_ps = smallp.tile([1, 512], F32, tag="ssq")
                for mi in range(4):
                    m = base + mi
                    pq = mmp.tile([128, 512], F32, tag="mm")
                    for k in range(KT):
                        nc.tensor.matmul(
                            pq, w_tiles[k][:, m * 128:(m + 1) * 128], xt[k],
                            start=(k == 0), stop=(k == KT - 1))
                    sq = sqp.tile([128, 512], BF16, tag="sq")
                    nc.scalar.square(sq, pq)
                    nc.tensor.matmul(ssq_ps, ones_col_b, sq,
                                     start=(mi == 0), stop=(mi == 3))
                    ncol = (qn_t if ti == 0 else kn_t)[:, mi:mi + 1]
                    nc.scalar.mul(qk_tiles[m][:, n * 512:(n + 1) * 512], pq, ncol)
                ssq_s = sqp.tile([1, 512], F32, tag="invd", name="ssq_s", bufs=2)
                nc.scalar.copy(ssq_s, ssq_ps)
                nc.sync.dma_start(
                    out=cc_in[ti:ti + 1, n * 512:(n + 1) * 512], in_=ssq_s)

            if n < 2:                               # v-proj inline (x resident)
                for tbl in range(4):
                    tb = n * 4 + tbl
                    pv = mmp.tile([128, 512], F32, tag="mm")
                    for k in range(KT):
                        nc.tensor.matmul(
                            pv, xt[k][:, tbl * 128:(tbl + 1) * 128],
                            w_tiles[k][:, 2 * DL:3 * DL],
                            start=(k == 0), stop=(k == KT - 1))
                    nc.vector.tensor_copy(v_tiles[tb], pv)
            else:                                   # keep x for post-AR v-proj
                saved_xt[n] = xt

        # ---------- phase 2: allreduce sumsq; v-proj of tiles 2,3 runs under
        # it using the still-resident x tiles (xpool bufs=2 holds both) ------
        nc.gpsimd.collective_compute(
            "AllReduce", mybir.AluOpType.add, replica_groups=GROUPS,
            ins=[cc_in[:, :]], outs=[cc_out[:, :]])

        for n in (2, 3):
            xt = saved_xt[n]
            for tbl in range(4):
                tb = n * 4 + tbl
                pv = mmp.tile([128, 512], F32, tag="mm")
                for k in range(KT):
                    nc.tensor.matmul(
                        pv, xt[k][:, tbl * 128:(tbl + 1) * 128],
                        w_tiles[k][:, 2 * DL:3 * DL],
                        start=(k == 0), stop=(k == KT - 1))
                nc.vector.tensor_copy(v_tiles[tb], pv)

        # w_out chunks reuse the w_in tag slots (the [128, 512] column slice
        # fits inside the [128, 1536] slot; tag reuse serializes each load
        # behind that chunk's last qkv/v matmul)
        w_out_tiles = []
        for k in range(KT):
            wo = weights.tile([128, DL], BF16, tag=f"w{k}", name=f"wo{k}")
            nc.sync.dma_start(out=wo, in_=w_outT[k * 128:(k + 1) * 128, :])
            w_out_tiles.append(wo)

        # ---------- phase 3: inv_rms + rope tables + rope ----------
        inv_t = []
        for t in range(2):
            tot = persist.tile([1, S], F32, tag=f"tot{t}", name=f"tot{t}")
            nc.sync.dma_start(out=tot, in_=cc_out[t:t + 1, :])
            nc.scalar.activation(tot, tot, mybir.ActivationFunctionType.Sqrt,
                                 bias=eps_b, scale=1.0 / (NH * HD))
            nc.vector.reciprocal(tot, tot)
            inv_t.append(tot)

        ci = [persist.tile([128, S], BF16, tag=f"ci{t}", name=f"ci{t}")
              for t in range(2)]
        si = [persist.tile([128, S], BF16, tag=f"si{t}", name=f"si{t}")
              for t in range(2)]
        for t in range(2):
            for j in range(NT):
                sl = slice(j * 512, (j + 1) * 512)
                bc = mmp.tile([128, 512], F32, tag="mm")
                nc.tensor.matmul(bc, ones_row, inv_t[t][:, sl],
                                 start=True, stop=True)
                cs = ropet.tile([128, 512], BF16, tag="rt", name="cs")
                nc.sync.dma_start(out=cs, in_=c128[:, sl])
                nc.vector.tensor_mul(ci[t][:, sl], cs, bc)
                ss = ropet.tile([128, 512], BF16, tag="rt2", name="ss")
                nc.sync.dma_start(out=ss, in_=s128[:, sl])
                nc.vector.tensor_mul(si[t][:, sl], ss, bc)

        # rope in place: q' = q*ci + rot(q)*si  (rot = half-swap via DMA,
        # si carries the [-sin; sin] signs)
        for j in range(NT):
            for m in range(8):
                t = 0 if m < 4 else 1
                qk = qk_tiles[m]
                sl = slice(j * 512, (j + 1) * 512)
                rq = rqp.tile([128, 512], BF16, tag="rq", name="rq")
                nc.sync.dma_start(out=rq[0:64, :], in_=qk[64:128, sl])
                nc.sync.dma_start(out=rq[64:128, :], in_=qk[0:64, sl])
                nc.vector.tensor_mul(qk[:, sl], qk[:, sl], ci[t][:, sl])
                nc.vector.tensor_mul(rq, rq, si[t][:, sl])
                nc.vector.tensor_add(qk[:, sl], qk[:, sl], rq)

        # ---------- phase 4+5: attention (j outer) + A2A + out proj per tile -
        for j in range(NT):
            attn_j = []
            for h in range(NHL):
                kt_h, qt_h = qk_tiles[4 + h], qk_tiles[h]
                pv_ps = accp.tile([128, 512], F32, tag="pv")
                den_ps = smallp.tile([1, 512], F32, tag="ssq")
                nb = 4 * j + 4
                for b in range(nb):
                    r = b - 4 * j
                    q_off = max(r, 0) * 128
                    w = 512 - q_off
                    s_ps = mmp.tile([128, 512], F32, tag="mm")
                    nc.tensor.matmul(
                        s_ps[:, :w], kt_h[:, b * 128:(b + 1) * 128],
                        qt_h[:, j * 512 + q_off:(j + 1) * 512],
                        start=True, stop=True)
                    ex = expp.tile([128, 512], BF16, tag="exp")
                    nc.scalar.activation(ex[:, :w], s_ps[:, :w],
                                         mybir.ActivationFunctionType.Exp,
                                         bias=zeros_b, scale=SCALE)
                    if r >= 0:
                        nc.vector.tensor_mul(ex[:, 0:128], ex[:, 0:128], mask_t)
                    nc.tensor.matmul(
                        pv_ps[:, q_off:512],
                        v_tiles[b][:, h * 128:(h + 1) * 128], ex[:, :w],
                        start=(b == 0), stop=(b == nb - 1))
                    nc.tensor.matmul(
                        den_ps[0:1, q_off:512], ones_col_b, ex[:, :w],
                        start=(b == 0), stop=(b == nb - 1))
                inv_d = sqp.tile([1, 512], BF16, tag="invd", bufs=2)
                with nc.allow_low_precision(reason="softmax denom bcast in bf16"):
                    nc.vector.reciprocal(inv_d, den_ps)
                bc = mmp.tile([128, 512], F32, tag="mm")
                nc.tensor.matmul(bc, ones_row_b, inv_d, start=True, stop=True)
                bc_sb = sqp.tile([128, 512], F32, tag="bcsb", name="bc_sb",
                                 bufs=1)
                nc.scalar.copy(bc_sb, bc)
                at = attnp.tile([128, 512], BF16, tag=f"at{h}", name=f"at{h}")
                nc.vector.tensor_mul(at, pv_ps, bc_sb)
                attn_j.append(at)

            # all-gather the tile's attention outputs across the group: shard
            # i of ag_out[j] = rank i's 512 head-dims (global flat dims
            # i*512..(i+1)*512) for all 512 tokens of the tile.
            for h in range(NHL):
                nc.sync.dma_start(
                    out=ag_in[j, h * 128:(h + 1) * 128, :], in_=attn_j[h])
            nc.gpsimd.collective_compute(
                "AllGather", mybir.AluOpType.bypass, replica_groups=GROUPS,
                ins=[ag_in[j]], outs=[ag_out[j]])

            # full-contraction out proj for MY 512 output-feature columns
            att_g = agp.tile([128, KT * 512], BF16, tag="ag", name="att_g")
            for k in range(KT):
                nc.sync.dma_start(out=att_g[:, k * 512:(k + 1) * 512],
                                  in_=ag_out[j, k * 128:(k + 1) * 128, :])
            for tb in range(4):
                po = mmp.tile([128, 512], F32, tag="mm")
                for k in range(KT):
                    nc.tensor.matmul(
                        po,
                        att_g[:, k * 512 + tb * 128:k * 512 + (tb + 1) * 128],
                        w_out_tiles[k],
                        start=(k == 0), stop=(k == KT - 1))
                fo = outp.tile([128, 512], F32, tag="fo", name="fo")
                nc.scalar.copy(fo, po)
                nc.sync.dma_start(
                    out=out[j * 512 + tb * 128:j * 512 + (tb + 1) * 128, :],
                    in_=fo)

    nc.finalize()
    return nc


def make_in_maps(x, w_in, w_out, q_norm_w, k_norm_w):
    x = np.asarray(x, np.float32)
    w_in = np.asarray(w_in, np.float32)
    w_out = np.asarray(w_out, np.float32)
    q_norm_w = np.asarray(q_norm_w, np.float32)
    k_norm_w = np.asarray(k_norm_w, np.float32)

    half = HD // 2
    inv_freq = 1.0 / (ROPE_BASE ** (np.arange(half, dtype=np.float32) / half))
    pos = np.arange(S, dtype=np.float32)
    ang = pos[:, None] * inv_freq[None, :]              # [S, 64]
    cos = np.cos(ang).T                                 # [64, S]
    sin = np.sin(ang).T
    c128 = np.concatenate([cos, cos], axis=0).astype(NPBF16)   # [128, S]
    s128 = np.concatenate([-sin, sin], axis=0).astype(NPBF16)  # rotate-half signs
    maskT = (np.arange(128)[:, None] <= np.arange(128)[None, :]).astype(NPBF16)

    in_maps = []
    for c in range(NCORES):
        b, hg = c // TPG, c % TPG
        rows = np.concatenate([
            w_in[hg * DL:(hg + 1) * DL],
            w_in[NH * HD + hg * DL:NH * HD + (hg + 1) * DL],
            w_in[2 * NH * HD + hg * DL:2 * NH * HD + (hg + 1) * DL],
        ], axis=0)                                      # [1536, HID]
        in_maps.append({
            "xT": np.ascontiguousarray(x[b].T).astype(NPBF16),
            "w_inT": np.ascontiguousarray(rows.T).astype(NPBF16),
            "w_outT": np.ascontiguousarray(
                w_out[hg * DL:(hg + 1) * DL, :].T).astype(NPBF16),
            "qn": np.ascontiguousarray(
                q_norm_w[hg * DL:(hg + 1) * DL].reshape(4, 128).T),
            "kn": np.ascontiguousarray(
                k_norm_w[hg * DL:(hg + 1) * DL].reshape(4, 128).T),
            "c128": c128, "s128": s128, "maskT": maskT,
        })
    return in_maps


def assemble(results):
    """results[c] is [S, 512]: all tokens of batch c//4, output-feature
    columns (c%4)*512..(c%4+1)*512."""
    outp = np.empty((B, S, HID), np.float32)
    for c in range(NCORES):
        b, hg = c // TPG, c % TPG
        outp[b, :, hg * DL:(hg + 1) * DL] = np.asarray(results[c], np.float32)
    return outp


def kernel(x, w_in, w_out, q_norm_w, k_norm_w, trace=False):
    global LAST_EXEC_NS, _CACHED_NC
    if _CACHED_NC is None:
        _CACHED_NC = build_nc()
    nc = _CACHED_NC
    in_maps = make_in_maps(x, w_in, w_out, q_norm_w, k_norm_w)
    res = run_bass_kernel_spmd(nc, in_maps, list(range(NCORES)), trace=trace)
    LAST_EXEC_NS = res.exec_time_ns
    return assemble([res.results[c]["out"] for c in range(NCORES)])



# revision 23
# speedup vs baseline: 1.3621x; 1.1468x over previous
"""Trainium2 Bass kernel for nn_CustomAttn: qkv proj + flat-axis qk-RMSnorm +
RoPE + causal attention + out proj, tensor-parallel over heads (Megatron-style)
with data-parallel batch, on 8 NeuronCores.

Mesh: core c -> batch b = c // 4, head-group hg = c % 4 (heads hg*4 .. hg*4+3).
Groups [[0,1,2,3],[4,5,6,7]]: AllReduce for the qk-norm sum-of-squares (the
norm is over the flat 2048-dim axis, i.e. across all 16 heads), and per
512-token-tile AllGather of the attention outputs (each rank contributes its
512 head-dims; 4x less effective wire time than ReduceScatter of 2 MB
partials).  After the gather every core holds the full [2048 dims, 512 tok]
attention block and runs the output projection over the full contraction for
ITS 512 output-feature columns only (w_out column-sliced by the host, so the
NEFF stays rank-symmetric), accumulating in f32.  Core c's `out` is
[S, 512] f32 = output features hg*512..(hg+1)*512 for all tokens of batch b.
"""

import sys

for p in ("/opt/trn_rl_repo",):
    if p not in sys.path:
        sys.path.insert(0, p)

import numpy as np
import ml_dtypes
from contextlib import ExitStack

import concourse.bass as bass
import concourse.bacc as bacc
from concourse.tile import TileContext
from concourse import mybir
from concourse.bass_utils import run_bass_kernel_spmd

BF16 = mybir.dt.bfloat16
F32 = mybir.dt.float32
NPBF16 = ml_dtypes.bfloat16

B, S, HID = 2, 2048, 2048
NH, HD = 16, 128
EPS = 1e-5
ROPE_BASE = 10000.0

NCORES = 8
TPG = 4                    # tensor-parallel group size
NHL = NH // TPG            # 4 local heads
DL = NHL * HD              # 512 local q/k/v dims
KT = HID // 128            # 16 contraction chunks
NT = S // 512              # 4 token tiles of 512
TB = S // 128              # 16 token blocks of 128
GROUPS = [[0, 1, 2, 3], [4, 5, 6, 7]]
SCALE = 1.0 / float(np.sqrt(HD))

LAST_EXEC_NS = None
_CACHED_NC = None


def build_nc():
    nc = bacc.Bacc(num_devices=NCORES)

    xT = nc.declare_dram_parameter("xT", [HID, S], BF16, isOutput=False)
    w_inT = nc.declare_dram_parameter("w_inT", [HID, 3 * DL], BF16, isOutput=False)
    w_outT = nc.declare_dram_parameter("w_outT", [NH * HD, DL], BF16, isOutput=False)
    qn = nc.declare_dram_parameter("qn", [128, 4], F32, isOutput=False)
    kn = nc.declare_dram_parameter("kn", [128, 4], F32, isOutput=False)
    c128 = nc.declare_dram_parameter("c128", [128, S], BF16, isOutput=False)
    s128 = nc.declare_dram_parameter("s128", [128, S], BF16, isOutput=False)
    maskT = nc.declare_dram_parameter("maskT", [128, 128], BF16, isOutput=False)
    out = nc.declare_dram_parameter("out", [S, DL], F32, isOutput=True)

    cc_in_h = [nc.dram_tensor(f"cc_in{i}", [2, S // 2], F32) for i in range(2)]
    cc_out_h = [nc.dram_tensor(f"cc_out{i}", [2, S // 2], F32) for i in range(2)]
    ag_in = nc.dram_tensor("ag_in", [NT, DL, 512], BF16)
    ag_out = nc.dram_tensor("ag_out", [NT, NH * HD, 512], BF16)

    with TileContext(nc) as tc, ExitStack() as ctx:
        consts = ctx.enter_context(tc.tile_pool(name="consts", bufs=1))
        weights = ctx.enter_context(tc.tile_pool(name="weights", bufs=1))
        persist = ctx.enter_context(tc.tile_pool(name="persist", bufs=1))
        xpool = ctx.enter_context(tc.tile_pool(name="xpool", bufs=2))
        sqp = ctx.enter_context(tc.tile_pool(name="sqp", bufs=2))
        mmp = ctx.enter_context(tc.tile_pool(name="mmp", bufs=3, space="PSUM"))
        accp = ctx.enter_context(tc.tile_pool(name="accp", bufs=2, space="PSUM"))
        smallp = ctx.enter_context(tc.tile_pool(name="smallp", bufs=2, space="PSUM"))
        ropet = ctx.enter_context(tc.tile_pool(name="ropet", bufs=1))
        rqp = ctx.enter_context(tc.tile_pool(name="rqp", bufs=2))
        attnp = ctx.enter_context(tc.tile_pool(name="attnp", bufs=2))
        expp = ctx.enter_context(tc.tile_pool(name="expp", bufs=4))
        agp = ctx.enter_context(tc.tile_pool(name="agp", bufs=1))
        outp = ctx.enter_context(tc.tile_pool(name="outp", bufs=2))

        # --- constants ---
        ones_col = consts.tile([128, 1], F32)          # lhsT for partition-sum (f32)
        nc.vector.memset(ones_col, 1.0)
        ones_col_b = consts.tile([128, 1], BF16)       # bf16 ones for denom/ssq
        nc.vector.memset(ones_col_b, 1.0)
        ones_row = consts.tile([1, 128], F32)          # lhsT for partition bcast
        nc.vector.memset(ones_row, 1.0)
        ones_row_b = consts.tile([1, 128], BF16)
        nc.vector.memset(ones_row_b, 1.0)
        qn_t = consts.tile([128, 4], F32)
        nc.sync.dma_start(out=qn_t, in_=qn[:, :])
        kn_t = consts.tile([128, 4], F32)
        nc.sync.dma_start(out=kn_t, in_=kn[:, :])
        mask_t = consts.tile([128, 128], BF16)
        nc.sync.dma_start(out=mask_t, in_=maskT[:, :])
        zeros_b = consts.tile([128, 1], F32)           # explicit bias for Exp
        nc.vector.memset(zeros_b, 0.0)
        eps_b = consts.tile([1, 1], F32)               # explicit bias for Sqrt
        nc.vector.memset(eps_b, EPS)

        # --- resident weights ---
        w_tiles = []
        for k in range(KT):
            wt = weights.tile([128, 3 * DL], BF16, tag=f"w{k}", name=f"w{k}")
            nc.sync.dma_start(out=wt, in_=w_inT[k * 128:(k + 1) * 128, :])
            w_tiles.append(wt)

        # q/k staging (rope applied in place later): m 0..3 = q chunks,
        # m 4..7 = k chunks, each [128 dims, S tokens] bf16
        qk_tiles = [persist.tile([128, S], BF16, tag=f"qk{m}", name=f"qk{m}")
                    for m in range(8)]
        # v in token-major layout: [128 tokens, 512 vdims] per token block
        v_tiles = [persist.tile([128, DL], BF16, tag=f"v{tb}", name=f"v{tb}")
                   for tb in range(TB)]

        # ---------- phase 1: q/k projection + sum-of-squares ----------
        saved_xt = {}
        for n in range(NT):
            xt = []
            for k in range(KT):
                t = xpool.tile([128, 512], BF16, tag=f"x{k}", name=f"x{k}")
                nc.sync.dma_start(
                    out=t, in_=xT[k * 128:(k + 1) * 128, n * 512:(n + 1) * 512])
                xt.append(t)

            for ti, base in ((0, 0), (1, 4)):       # q then k chunks
                ssq_ps = smallp.tile([1, 512], F32, tag="ssq")
                for mi in range(4):
                    m = base + mi
                    pq = mmp.tile([128, 512], F32, tag="mm")
                    for k in range(KT):
                        nc.tensor.matmul(
                            pq, w_tiles[k][:, m * 128:(m + 1) * 128], xt[k],
                            start=(k == 0), stop=(k == KT - 1))
                    sq = sqp.tile([128, 512], BF16, tag="sq")
                    nc.scalar.square(sq, pq)
                    nc.tensor.matmul(ssq_ps, ones_col_b, sq,
                                     start=(mi == 0), stop=(mi == 3))
                    ncol = (qn_t if ti == 0 else kn_t)[:, mi:mi + 1]
                    nc.scalar.mul(qk_tiles[m][:, n * 512:(n + 1) * 512], pq, ncol)
                ssq_s = sqp.tile([1, 512], F32, tag="invd", name="ssq_s", bufs=2)
                nc.scalar.copy(ssq_s, ssq_ps)
                nc.sync.dma_start(
                    out=cc_in_h[n // 2][ti:ti + 1,
                                        (n % 2) * 512:(n % 2 + 1) * 512],
                    in_=ssq_s)

            if n < 2:                               # v-proj inline (x resident)
                for tbl in range(4):
                    tb = n * 4 + tbl
                    pv = mmp.tile([128, 512], F32, tag="mm")
                    for k in range(KT):
                        nc.tensor.matmul(
                            pv, xt[k][:, tbl * 128:(tbl + 1) * 128],
                            w_tiles[k][:, 2 * DL:3 * DL],
                            start=(k == 0), stop=(k == KT - 1))
                    nc.vector.tensor_copy(v_tiles[tb], pv)
            else:                                   # keep x for post-AR v-proj
                saved_xt[n] = xt

        # ---------- phase 2: allreduce sumsq in two halves.  AR#1 (tiles
        # 0,1) fires as soon as their ssq lands and hides under the tile-2/3
        # projections; AR#2 hides under the post-AR v-proj ----------
        nc.gpsimd.collective_compute(
            "AllReduce", mybir.AluOpType.add, replica_groups=GROUPS,
            ins=[cc_in_h[0][:, :]], outs=[cc_out_h[0][:, :]])
        nc.gpsimd.collective_compute(
            "AllReduce", mybir.AluOpType.add, replica_groups=GROUPS,
            ins=[cc_in_h[1][:, :]], outs=[cc_out_h[1][:, :]])

        for n in (2, 3):
            xt = saved_xt[n]
            for tbl in range(4):
                tb = n * 4 + tbl
                pv = mmp.tile([128, 512], F32, tag="mm")
                for k in range(KT):
                    nc.tensor.matmul(
                        pv, xt[k][:, tbl * 128:(tbl + 1) * 128],
                        w_tiles[k][:, 2 * DL:3 * DL],
                        start=(k == 0), stop=(k == KT - 1))
                nc.vector.tensor_copy(v_tiles[tb], pv)

        # w_out chunks reuse the w_in tag slots (the [128, 512] column slice
        # fits inside the [128, 1536] slot; tag reuse serializes each load
        # behind that chunk's last qkv/v matmul)
        w_out_tiles = []
        for k in range(KT):
            wo = weights.tile([128, DL], BF16, tag=f"w{k}", name=f"wo{k}")
            nc.sync.dma_start(out=wo, in_=w_outT[k * 128:(k + 1) * 128, :])
            w_out_tiles.append(wo)

        # ---------- phase 3: inv_rms + rope tables + rope (per AR half, so
        # tiles 0,1 proceed without waiting for AR#2) ----------
        inv_t = []
        for t in range(2):
            tot = persist.tile([1, S], F32, tag=f"tot{t}", name=f"tot{t}")
            for hf in range(2):
                sl = slice(hf * (S // 2), (hf + 1) * (S // 2))
                nc.sync.dma_start(out=tot[:, sl], in_=cc_out_h[hf][t:t + 1, :])
                nc.scalar.activation(tot[:, sl], tot[:, sl],
                                     mybir.ActivationFunctionType.Sqrt,
                                     bias=eps_b, scale=1.0 / (NH * HD))
                nc.vector.reciprocal(tot[:, sl], tot[:, sl])
            inv_t.append(tot)

        ci = [persist.tile([128, S], BF16, tag=f"ci{t}", name=f"ci{t}")
              for t in range(2)]
        si = [persist.tile([128, S], BF16, tag=f"si{t}", name=f"si{t}")
              for t in range(2)]
        for t in range(2):
            for j in range(NT):
                sl = slice(j * 512, (j + 1) * 512)
                bc = mmp.tile([128, 512], F32, tag="mm")
                nc.tensor.matmul(bc, ones_row, inv_t[t][:, sl],
                                 start=True, stop=True)
                cs = ropet.tile([128, 512], BF16, tag="rt", name="cs")
                nc.sync.dma_start(out=cs, in_=c128[:, sl])
                nc.vector.tensor_mul(ci[t][:, sl], cs, bc)
                ss = ropet.tile([128, 512], BF16, tag="rt2", name="ss")
                nc.sync.dma_start(out=ss, in_=s128[:, sl])
                nc.vector.tensor_mul(si[t][:, sl], ss, bc)

        # rope in place: q' = q*ci + rot(q)*si  (rot = half-swap via DMA,
        # si carries the [-sin; sin] signs)
        for j in range(NT):
            for m in range(8):
                t = 0 if m < 4 else 1
                qk = qk_tiles[m]
                sl = slice(j * 512, (j + 1) * 512)
                rq = rqp.tile([128, 512], BF16, tag="rq", name="rq")
                nc.sync.dma_start(out=rq[0:64, :], in_=qk[64:128, sl])
                nc.sync.dma_start(out=rq[64:128, :], in_=qk[0:64, sl])
                nc.vector.tensor_mul(qk[:, sl], qk[:, sl], ci[t][:, sl])
                nc.vector.tensor_mul(rq, rq, si[t][:, sl])
                nc.vector.tensor_add(qk[:, sl], qk[:, sl], rq)

        # ---------- phase 4+5: attention (j outer) + A2A + out proj per tile -
        for j in range(NT):
            attn_j = []
            for h in range(NHL):
                kt_h, qt_h = qk_tiles[4 + h], qk_tiles[h]
                pv_ps = accp.tile([128, 512], F32, tag="pv")
                den_ps = smallp.tile([1, 512], F32, tag="ssq")
                nb = 4 * j + 4
                for b in range(nb):
                    r = b - 4 * j
                    q_off = max(r, 0) * 128
                    w = 512 - q_off
                    s_ps = mmp.tile([128, 512], F32, tag="mm")
                    nc.tensor.matmul(
                        s_ps[:, :w], kt_h[:, b * 128:(b + 1) * 128],
                        qt_h[:, j * 512 + q_off:(j + 1) * 512],
                        start=True, stop=True)
                    ex = expp.tile([128, 512], BF16, tag="exp")
                    nc.scalar.activation(ex[:, :w], s_ps[:, :w],
                                         mybir.ActivationFunctionType.Exp,
                                         bias=zeros_b, scale=SCALE)
                    if r >= 0:
                        nc.vector.tensor_mul(ex[:, 0:128], ex[:, 0:128], mask_t)
                    nc.tensor.matmul(
                        pv_ps[:, q_off:512],
                        v_tiles[b][:, h * 128:(h + 1) * 128], ex[:, :w],
                        start=(b == 0), stop=(b == nb - 1))
                    nc.tensor.matmul(
                        den_ps[0:1, q_off:512], ones_col_b, ex[:, :w],
                        start=(b == 0), stop=(b == nb - 1))
                inv_d = sqp.tile([1, 512], BF16, tag="invd", bufs=2)
                with nc.allow_low_precision(reason="softmax denom bcast in bf16"):
                    nc.vector.reciprocal(inv_d, den_ps)
                bc = mmp.tile([128, 512], F32, tag="mm")
                nc.tensor.matmul(bc, ones_row_b, inv_d, start=True, stop=True)
                bc_sb = sqp.tile([128, 512], F32, tag="bcsb", name="bc_sb",
                                 bufs=1)
                nc.scalar.copy(bc_sb, bc)
                at = attnp.tile([128, 512], BF16, tag=f"at{h}", name=f"at{h}")
                nc.vector.tensor_mul(at, pv_ps, bc_sb)
                attn_j.append(at)

            # all-gather the tile's attention outputs across the group: shard
            # i of ag_out[j] = rank i's 512 head-dims (global flat dims
            # i*512..(i+1)*512) for all 512 tokens of the tile.
            for h in range(NHL):
                nc.sync.dma_start(
                    out=ag_in[j, h * 128:(h + 1) * 128, :], in_=attn_j[h])
            nc.gpsimd.collective_compute(
                "AllGather", mybir.AluOpType.bypass, replica_groups=GROUPS,
                ins=[ag_in[j]], outs=[ag_out[j]])

            # full-contraction out proj for MY 512 output-feature columns
            att_g = agp.tile([128, KT * 512], BF16, tag="ag", name="att_g")
            for k in range(KT):
                nc.sync.dma_start(out=att_g[:, k * 512:(k + 1) * 512],
                                  in_=ag_out[j, k * 128:(k + 1) * 128, :])
            for tb in range(4):
                po = mmp.tile([128, 512], F32, tag="mm")
                for k in range(KT):
                    nc.tensor.matmul(
                        po,
                        att_g[:, k * 512 + tb * 128:k * 512 + (tb + 1) * 128],
                        w_out_tiles[k],
                        start=(k == 0), stop=(k == KT - 1))
                fo = outp.tile([128, 512], F32, tag="fo", name="fo")
                nc.scalar.copy(fo, po)
                nc.sync.dma_start(
                    out=out[j * 512 + tb * 128:j * 512 + (tb + 1) * 128, :],
                    in_=fo)

    nc.finalize()
    return nc


def make_in_maps(x, w_in, w_out, q_norm_w, k_norm_w):
    x = np.asarray(x, np.float32)
    w_in = np.asarray(w_in, np.float32)
    w_out = np.asarray(w_out, np.float32)
    q_norm_w = np.asarray(q_norm_w, np.float32)
    k_norm_w = np.asarray(k_norm_w, np.float32)

    half = HD // 2
    inv_freq = 1.0 / (ROPE_BASE ** (np.arange(half, dtype=np.float32) / half))
    pos = np.arange(S, dtype=np.float32)
    ang = pos[:, None] * inv_freq[None, :]              # [S, 64]
    cos = np.cos(ang).T                                 # [64, S]
    sin = np.sin(ang).T
    c128 = np.concatenate([cos, cos], axis=0).astype(NPBF16)   # [128, S]
    s128 = np.concatenate([-sin, sin], axis=0).astype(NPBF16)  # rotate-half signs
    maskT = (np.arange(128)[:, None] <= np.arange(128)[None, :]).astype(NPBF16)

    in_maps = []
    for c in range(NCORES):
        b, hg = c // TPG, c % TPG
        rows = np.concatenate([
            w_in[hg * DL:(hg + 1) * DL],
            w_in[NH * HD + hg * DL:NH * HD + (hg + 1) * DL],
            w_in[2 * NH * HD + hg * DL:2 * NH * HD + (hg + 1) * DL],
        ], axis=0)                                      # [1536, HID]
        in_maps.append({
            "xT": np.ascontiguousarray(x[b].T).astype(NPBF16),
            "w_inT": np.ascontiguousarray(rows.T).astype(NPBF16),
            "w_outT": np.ascontiguousarray(
                w_out[hg * DL:(hg + 1) * DL, :].T).astype(NPBF16),
            "qn": np.ascontiguousarray(
                q_norm_w[hg * DL:(hg + 1) * DL].reshape(4, 128).T),
            "kn": np.ascontiguousarray(
                k_norm_w[hg * DL:(hg + 1) * DL].reshape(4, 128).T),
            "c128": c128, "s128": s128, "maskT": maskT,
        })
    return in_maps


def assemble(results):
    """results[c] is [S, 512]: all tokens of batch c//4, output-feature
    columns (c%4)*512..(c%4+1)*512."""
    outp = np.empty((B, S, HID), np.float32)
    for c in range(NCORES):
        b, hg = c // TPG, c % TPG
        outp[b, :, hg * DL:(hg + 1) * DL] = np.asarray(results[c], np.float32)
    return outp


def kernel(x, w_in, w_out, q_norm_w, k_norm_w, trace=False):
    global LAST_EXEC_NS, _CACHED_NC
    if _CACHED_NC is None:
        _CACHED_NC = build_nc()
    nc = _CACHED_NC
    in_maps = make_in_maps(x, w_in, w_out, q_norm_w, k_norm_w)
    res = run_bass_kernel_spmd(nc, in_maps, list(range(NCORES)), trace=trace)
    LAST_EXEC_NS = res.exec_time_ns
    return assemble([res.results[c]["out"] for c in range(NCORES)])

